# revision 10
# baseline (speedup 1.0000x reference)
"""Self-contained 8-core Trainium2 Bass kernel for a 2-layer GAT + linear classifier.

Strategy (dst-sharded 1D graph parallelism):
  - Host: add self-loops, compute in-degrees, sort nodes by degree (desc),
    deal round-robin to 8 cores.  Each core owns 12500 dst nodes; its nodes
    are degree-sorted so 128-node tiles have near-uniform degree.  All
    edge-index-derived tables (gather offset tables, per-tile slot counts)
    are precomputed on the host and baked into the program/inputs.
  - Device, per layer: h' = h @ W and s = h'@a_s are computed per owner shard;
    rows [h' | s] are AllGathered into a full HBM table (100001 x 65, last row
    is a sentinel with h=0, s=-1e4 used for padding slots).  Edges of each dst
    tile are gathered with one indirect DMA (260B per edge slot) into SBUF as
    [128 nodes x D_t slots x 65].  Segment softmax runs along the free axis
    (ACT for exp, DVE for max/recip), the p-weighted aggregation multiplies
    in-place on DVE (broadcast AP) and reduces over slots with PE
    identity-matmul accumulation in PSUM.
  - Output: classifier per tile, unpermuted on the host.
"""

import os
import sys
import types
from dataclasses import dataclass, field

import numpy as np

P = 128


@dataclass
class GatCfg:
    n: int = 100000
    in_dim: int = 128
    hid: int = 64
    classes: int = 40
    cores: int = 8
    slope: float = 0.2
    s_pad: float = -1.0e4
    group_slots: int = 160
    # debug staging: 1=phaseA only, 2=+AG0, 3=+edge L0, 4=full
    stage: int = 4
    use_pe_reduce: bool = False

    @property
    def shard(self):
        assert self.n % self.cores == 0
        return self.n // self.cores

    @property
    def nt(self):
        return (self.shard + P - 1) // P

    @property
    def tab(self):
        return self.n + 1


CFG = GatCfg()


def _ensure_profile_hook():
    """Synthesize antenv.axon_hooks so trace=True can capture NTFF under axon."""
    if "antenv.axon_hooks" in sys.modules:
        return
    try:
        import antenv
        mod = types.ModuleType("antenv.axon_hooks")
        mod._hook = None
        def _set(h):
            mod._hook = h
        def _get():
            return mod._hook
        mod.set_axon_ntff_profile_hook = _set
        mod.get_axon_ntff_profile_hook = _get
        sys.modules["antenv.axon_hooks"] = mod
        antenv.axon_hooks = mod
        from trn_agent_boot.trn_boot import _ntff_profile_via_ctypes
        _set(_ntff_profile_via_ctypes("/opt/axon/libaxon_pjrt.so"))
    except Exception:
        pass


# --------------------------------------------------------------------------
# Host preprocessing: sharding, tiling, gather tables
# --------------------------------------------------------------------------

@dataclass
class Prep:
    perms: list                       # per core: global node ids in local order
    D: np.ndarray                     # [nt] slots per node for each tile
    rows_t: list                      # [nt] rows per tile
    groups: list                      # (t0, n_tiles, S, rows, idx_off)
    idx_cores: list                   # per core: flat int32 gather tables
    tot_idx: int = 0


def preprocess(edge_index, cfg: GatCfg) -> Prep:
    N, NC, GS = cfg.n, cfg.cores, cfg.group_slots
    shard = cfg.shard
    src = np.asarray(edge_index[0]).astype(np.int64)
    dst = np.asarray(edge_index[1]).astype(np.int64)
    loop = np.arange(N, dtype=np.int64)
    src_all = np.concatenate([src, loop])
    dst_all = np.concatenate([dst, loop])
    deg = np.bincount(dst_all, minlength=N).astype(np.int64)
    order = np.argsort(dst_all, kind="stable")
    srcs_by_dst = src_all[order]
    rowptr = np.zeros(N + 1, np.int64)
    np.cumsum(deg, out=rowptr[1:])

    rank_order = np.argsort(-deg, kind="stable")
    table_row = np.empty(N, np.int64)
    rr = np.arange(N)
    table_row[rank_order] = (rr % NC) * shard + rr // NC
    perms = [rank_order[c::NC] for c in range(NC)]

    nt = cfg.nt
    rows_t = [min(P, shard - t * P) for t in range(nt)]
    D = np.zeros(nt, np.int64)
    for c in range(NC):
        dc = deg[perms[c]]
        for t in range(nt):
            D[t] = max(D[t], dc[t * P:t * P + rows_t[t]].max())

    groups = []
    t = 0
    idx_off = 0
    while t < nt:
        if rows_t[t] < P:
            groups.append((t, 1, int(D[t]), rows_t[t], idx_off))
            idx_off += rows_t[t] * int(D[t])
            t += 1
            continue
        S = 0
        t0 = t
        while t < nt and rows_t[t] == P and (S == 0 or S + D[t] <= GS):
            S += int(D[t])
            t += 1
        groups.append((t0, t - t0, S, P, idx_off))
        idx_off += P * S
    tot_idx = idx_off

    SENT = N
    idx_cores = []
    for c in range(NC):
        parts = []
        for (t0, ntl, S, rows, _off) in groups:
            arr = np.full((rows, S), SENT, np.int32)
            off = 0
            for t2 in range(t0, t0 + ntl):
                Dt = int(D[t2])
                nodes = perms[c][t2 * P:t2 * P + rows]
                degs = deg[nodes]
                starts = rowptr[nodes]
                pos = starts[:, None] + np.arange(Dt)[None, :]
                mask = np.arange(Dt)[None, :] < degs[:, None]
                block = np.full((rows, Dt), SENT, np.int64)
                block[mask] = table_row[srcs_by_dst[pos[mask]]]
                arr[:, off:off + Dt] = block.astype(np.int32)
                off += Dt
            parts.append(arr.reshape(-1))
        idx_cores.append(np.ascontiguousarray(np.concatenate(parts), dtype=np.int32))
        assert idx_cores[-1].size == tot_idx
    return Prep(perms=perms, D=D, rows_t=rows_t, groups=groups,
                idx_cores=idx_cores, tot_idx=tot_idx)


# --------------------------------------------------------------------------
# Device program
# --------------------------------------------------------------------------

def build_program(cfg: GatCfg, pp: Prep, enable_asserts=False):
    import concourse.bass as bass
    import concourse.mybir as mybir
    import concourse.tile as tile
    from concourse import bacc

    f32 = mybir.dt.float32
    i32 = mybir.dt.int32
    A = mybir.AluOpType
    AF = mybir.ActivationFunctionType
    IN, H, CLS, NC = cfg.in_dim, cfg.hid, cfg.classes, cfg.cores
    HS = H + 1
    N, NT, TAB, shard = cfg.n, cfg.nt, cfg.tab, cfg.shard
    Dmax = int(pp.D.max())
    GSmax = max(g[2] for g in pp.groups)

    nc = bacc.Bacc("TRN2", target_bir_lowering=False, debug=False,
                   enable_asserts=enable_asserts, num_devices=NC)

    x_t = nc.dram_tensor("x_shard", [shard, IN], f32, kind="ExternalInput")
    idx_t = nc.dram_tensor("idx_flat", [pp.tot_idx], i32, kind="ExternalInput")
    W0_t = nc.dram_tensor("W0", [IN, H], f32, kind="ExternalInput")
    W1_t = nc.dram_tensor("W1", [H, H], f32, kind="ExternalInput")
    Wl_t = nc.dram_tensor("Wl", [H, CLS], f32, kind="ExternalInput")
    asb0_t = nc.dram_tensor("asb0", [P, H], f32, kind="ExternalInput")
    adb0_t = nc.dram_tensor("adb0", [P, H], f32, kind="ExternalInput")
    asb1_t = nc.dram_tensor("asb1", [P, H], f32, kind="ExternalInput")
    adb1_t = nc.dram_tensor("adb1", [P, H], f32, kind="ExternalInput")
    b0b_t = nc.dram_tensor("b0b", [P, H], f32, kind="ExternalInput")
    b1b_t = nc.dram_tensor("b1b", [P, H], f32, kind="ExternalInput")
    blb_t = nc.dram_tensor("blb", [P, CLS], f32, kind="ExternalInput")
    ident_t = nc.dram_tensor("ident", [P, P], f32, kind="ExternalInput")
    sent_t = nc.dram_tensor("sentrow", [1, HS], f32, kind="ExternalInput")
    y_t = nc.dram_tensor("y_out", [shard, CLS], f32, kind="ExternalOutput")

    ha0_loc = nc.dram_tensor("ha0_loc", [shard, HS], f32, kind="Internal")
    ha1_loc = nc.dram_tensor("ha1_loc", [shard, HS], f32, kind="Internal")
    ha0_full = nc.dram_tensor("ha0_full", [TAB, HS], f32, kind="Internal")
    ha1_full = nc.dram_tensor("ha1_full", [TAB, HS], f32, kind="Internal")

    with tile.TileContext(nc) as tc:
        with tc.tile_pool(name="const", bufs=1) as cp, \
             tc.tile_pool(name="gp", bufs=3) as gp, \
             tc.tile_pool(name="wp", bufs=3) as wp, \
             tc.tile_pool(name="ps", bufs=2, space="PSUM") as ps:

            def load_const(t, shape):
                s = cp.tile(shape, f32, tag=f"c_{t.name}")
                nc.sync.dma_start(s[:], t.ap())
                return s

            W0_s = load_const(W0_t, [IN, H])
            W1_s = load_const(W1_t, [H, H])
            Wl_s = load_const(Wl_t, [H, CLS])
            asb0_s = load_const(asb0_t, [P, H])
            adb0_s = load_const(adb0_t, [P, H])
            asb1_s = load_const(asb1_t, [P, H])
            adb1_s = load_const(adb1_t, [P, H])
            b0b_s = load_const(b0b_t, [P, H])
            b1b_s = load_const(b1b_t, [P, H])
            blb_s = load_const(blb_t, [P, CLS])
            ident_s = load_const(ident_t, [P, P])
            sent_s = load_const(sent_t, [1, HS])

            d0_all = cp.tile([P, NT], f32)
            d0s_all = cp.tile([P, NT], f32)
            d1_all = cp.tile([P, NT], f32)
            d1s_all = cp.tile([P, NT], f32)
            junk = cp.tile([P, H], f32)

            # sentinel rows of both tables
            nc.sync.dma_start(ha0_full.ap()[N:N + 1, :], sent_s[:])
            nc.sync.dma_start(ha1_full.ap()[N:N + 1, :], sent_s[:])

            # ---------------- phase A: h0' = x @ W0, s0, d0 ----------------
            for t in range(NT):
                rows = pp.rows_t[t]
                x_s = wp.tile([rows, IN], f32, tag="xs")
                nc.sync.dma_start(x_s[:], x_t.ap()[t * P:t * P + rows, :])
                xT_p = ps.tile([IN, P], f32, space="PSUM", tag="tp")
                nc.tensor.transpose(out=xT_p[:, :rows], in_=x_s[:],
                                    identity=ident_s[:rows, :rows])
                xT_s = wp.tile([IN, P], f32, tag="xts")
                nc.vector.tensor_copy(out=xT_s[:, :rows], in_=xT_p[:, :rows])
                h0_p = ps.tile([P, H], f32, space="PSUM", tag="mm")
                nc.tensor.matmul(out=h0_p[:rows, :], lhsT=xT_s[:, :rows],
                                 rhs=W0_s[:], start=True, stop=True)
                ha_tile = wp.tile([rows, HS], f32, tag="hat")
                nc.scalar.copy(ha_tile[:, :H], h0_p[:rows, :])
                nc.vector.tensor_tensor(out=junk[:rows, :], in0=h0_p[:rows, :],
                                        in1=asb0_s[:rows, :], op=A.mult)
                nc.vector.tensor_reduce(out=ha_tile[:, H:HS], in_=junk[:rows, :],
                                        axis=mybir.AxisListType.X, op=A.add)
                nc.vector.tensor_tensor(out=junk[:rows, :], in0=h0_p[:rows, :],
                                        in1=adb0_s[:rows, :], op=A.mult)
                nc.vector.tensor_reduce(out=d0_all[:rows, t:t + 1],
                                        in_=junk[:rows, :],
                                        axis=mybir.AxisListType.X, op=A.add)
                nc.sync.dma_start(ha0_loc.ap()[t * P:t * P + rows, :], ha_tile[:])
            nc.vector.tensor_scalar(d0s_all[:], d0_all[:], cfg.slope, None,
                                    op0=A.mult)

            if cfg.stage >= 2:
                nc.gpsimd.collective_compute(
                    "AllGather", A.bypass,
                    replica_groups=[list(range(NC))],
                    ins=[ha0_loc.ap()],
                    outs=[ha0_full.ap()[0:N, :]],
                )

            # ---------------- edge phase ----------------
            def edge_phase(table, d_all, ds_all, post):
                for (t0, ntl, S, rows, idx_off) in pp.groups:
                    idx_s = gp.tile([rows, S], i32, tag="idx")
                    nc.sync.dma_start(
                        idx_s[:],
                        idx_t.ap()[idx_off:idx_off + rows * S]
                        .rearrange("(p s) -> p s", s=S))
                    G = gp.tile([rows, S, HS], f32, tag="G")
                    # HW indirect DMA consumes ONE offset per partition per
                    # call (sim's multi-offset form diverges on silicon), so
                    # gather one slot-column (<=128 rows) at a time.
                    for j in range(S):
                        nc.gpsimd.indirect_dma_start(
                            out=G[:, j, :], out_offset=None,
                            in_=table.ap(),
                            in_offset=bass.IndirectOffsetOnAxis(
                                ap=idx_s[:, j:j + 1], axis=0))
                    off = 0
                    for t in range(t0, t0 + ntl):
                        Dt = int(pp.D[t])
                        Gt = G[:, off:off + Dt, :]
                        off += Dt
                        sG = Gt[:, :, H]
                        z = wp.tile([rows, Dt], f32, tag="z")
                        nc.scalar.activation(z[:], sG, AF.Identity,
                                             bias=d_all[:rows, t:t + 1], scale=1.0)
                        t02 = wp.tile([rows, Dt], f32, tag="t02")
                        nc.scalar.activation(t02[:], sG, AF.Identity,
                                             bias=ds_all[:rows, t:t + 1],
                                             scale=cfg.slope)
                        nc.vector.tensor_tensor(out=z[:], in0=z[:], in1=t02[:],
                                                op=A.max)
                        nm = wp.tile([rows, 1], f32, tag="nm")
                        nc.vector.tensor_reduce(out=nm[:], in_=z[:],
                                                axis=mybir.AxisListType.X,
                                                op=A.max, negate=True)
                        p_t = wp.tile([rows, Dt], f32, tag="pt")
                        den = wp.tile([rows, 1], f32, tag="den")
                        nc.scalar.activation(p_t[:], z[:], AF.Exp, bias=nm[:],
                                             scale=1.0, accum_out=den[:])
                        r_t = wp.tile([rows, 1], f32, tag="rt")
                        nc.vector.reciprocal(r_t[:], den[:])
                        nc.vector.tensor_tensor(
                            out=Gt[:, :, 0:H], in0=Gt[:, :, 0:H],
                            in1=p_t[:].to_broadcast([rows, Dt, H]), op=A.mult)
                        hagg = wp.tile([rows, H], f32, tag="hagg")
                        if cfg.use_pe_reduce:
                            agg_p = ps.tile([P, H], f32, space="PSUM", tag="agg")
                            for d in range(Dt):
                                nc.tensor.matmul(out=agg_p[:rows, :],
                                                 lhsT=ident_s[:rows, :rows],
                                                 rhs=Gt[:, d, 0:H],
                                                 start=(d == 0),
                                                 stop=(d == Dt - 1))
                            nc.vector.tensor_scalar(hagg[:], agg_p[:rows, :],
                                                    r_t[:], None, op0=A.mult)
                        else:
                            agg_s = wp.tile([rows, H], f32, tag="aggs")
                            nc.vector.tensor_reduce(
                                out=agg_s[:],
                                in_=Gt[:, :, 0:H].rearrange("p d f -> p f d"),
                                axis=mybir.AxisListType.X, op=A.add)
                            nc.vector.tensor_scalar(hagg[:], agg_s[:],
                                                    r_t[:], None, op0=A.mult)
                        bias_s = b0b_s if table is ha0_full else b1b_s
                        nc.vector.tensor_tensor(out=hagg[:], in0=hagg[:],
                                                in1=bias_s[:rows, :], op=A.add)
                        # ELU = relu(x) + min(exp(x)-1, 0)
                        ex = wp.tile([rows, H], f32, tag="ex")
                        nc.scalar.activation(ex[:], hagg[:], AF.Exp)
                        nc.vector.tensor_scalar(ex[:], ex[:], -1.0, 0.0,
                                                op0=A.add, op1=A.min)
                        rl = wp.tile([rows, H], f32, tag="rl")
                        nc.vector.tensor_scalar(rl[:], hagg[:], 0.0, None,
                                                op0=A.max)
                        h_t = wp.tile([rows, H], f32, tag="ht")
                        nc.vector.tensor_tensor(out=h_t[:], in0=rl[:], in1=ex[:],
                                                op=A.add)
                        post(t, h_t, rows)

            # ---------------- layer-0 post: h1' = elu_out @ W1, s1, d1 -----
            def post_l0(t, h_t, rows):
                hT_p = ps.tile([H, P], f32, space="PSUM", tag="tp")
                nc.tensor.transpose(out=hT_p[:, :rows], in_=h_t[:],
                                    identity=ident_s[:rows, :rows])
                hT_s = wp.tile([H, P], f32, tag="hts")
                nc.vector.tensor_copy(out=hT_s[:, :rows], in_=hT_p[:, :rows])
                h1_p = ps.tile([P, H], f32, space="PSUM", tag="mm")
                nc.tensor.matmul(out=h1_p[:rows, :], lhsT=hT_s[:, :rows],
                                 rhs=W1_s[:], start=True, stop=True)
                ha_tile = wp.tile([rows, HS], f32, tag="hat")
                nc.scalar.copy(ha_tile[:, :H], h1_p[:rows, :])
                nc.vector.tensor_tensor(out=junk[:rows, :], in0=h1_p[:rows, :],
                                        in1=asb1_s[:rows, :], op=A.mult)
                nc.vector.tensor_reduce(out=ha_tile[:, H:HS], in_=junk[:rows, :],
                                        axis=mybir.AxisListType.X, op=A.add)
                nc.vector.tensor_tensor(out=junk[:rows, :], in0=h1_p[:rows, :],
                                        in1=adb1_s[:rows, :], op=A.mult)
                nc.vector.tensor_reduce(out=d1_all[:rows, t:t + 1],
                                        in_=junk[:rows, :],
                                        axis=mybir.AxisListType.X, op=A.add)
                nc.sync.dma_start(ha1_loc.ap()[t * P:t * P + rows, :], ha_tile[:])

            if cfg.stage >= 3:
                edge_phase(ha0_full, d0_all, d0s_all, post_l0)
            if cfg.stage >= 4:
                nc.vector.tensor_scalar(d1s_all[:], d1_all[:], cfg.slope, None,
                                        op0=A.mult)
                nc.gpsimd.collective_compute(
                    "AllGather", A.bypass,
                    replica_groups=[list(range(NC))],
                    ins=[ha1_loc.ap()],
                    outs=[ha1_full.ap()[0:N, :]],
                )

            # ---------------- layer-1 post: classifier ----------------
            def post_l1(t, h_t, rows):
                hT_p = ps.tile([H, P], f32, space="PSUM", tag="tp")
                nc.tensor.transpose(out=hT_p[:, :rows], in_=h_t[:],
                                    identity=ident_s[:rows, :rows])
                hT_s = wp.tile([H, P], f32, tag="hts")
                nc.vector.tensor_copy(out=hT_s[:, :rows], in_=hT_p[:, :rows])
                y_p = ps.tile([P, CLS], f32, space="PSUM", tag="mm")
                nc.tensor.matmul(out=y_p[:rows, :], lhsT=hT_s[:, :rows],
                                 rhs=Wl_s[:], start=True, stop=True)
                y_s = wp.tile([rows, CLS], f32, tag="ys")
                nc.vector.tensor_tensor(out=y_s[:], in0=y_p[:rows, :],
                                        in1=blb_s[:rows, :], op=A.add)
                nc.sync.dma_start(y_t.ap()[t * P:t * P + rows, :], y_s[:])

            if cfg.stage >= 4:
                edge_phase(ha1_full, d1_all, d1s_all, post_l1)
            else:
                for t in range(NT):
                    rows = pp.rows_t[t]
                    nc.sync.dma_start(y_t.ap()[t * P:t * P + rows, :],
                                      blb_s[:rows, :])

    nc.compile()
    return nc


def make_in_maps(cfg: GatCfg, pp: Prep, x, W0, a_s0, a_d0, b0, W1, a_s1, a_d1,
                 b1, Wl, bl):
    x = np.asarray(x, np.float32)
    consts = dict(
        W0=np.ascontiguousarray(W0, np.float32),
        W1=np.ascontiguousarray(W1, np.float32),
        Wl=np.ascontiguousarray(Wl, np.float32),
        asb0=np.ascontiguousarray(np.tile(np.asarray(a_s0, np.float32)[None, :], (P, 1))),
        adb0=np.ascontiguousarray(np.tile(np.asarray(a_d0, np.float32)[None, :], (P, 1))),
        asb1=np.ascontiguousarray(np.tile(np.asarray(a_s1, np.float32)[None, :], (P, 1))),
        adb1=np.ascontiguousarray(np.tile(np.asarray(a_d1, np.float32)[None, :], (P, 1))),
        b0b=np.ascontiguousarray(np.tile(np.asarray(b0, np.float32)[None, :], (P, 1))),
        b1b=np.ascontiguousarray(np.tile(np.asarray(b1, np.float32)[None, :], (P, 1))),
        blb=np.ascontiguousarray(np.tile(np.asarray(bl, np.float32)[None, :], (P, 1))),
        ident=np.eye(P, dtype=np.float32),
        sentrow=np.ascontiguousarray(
            np.concatenate([np.zeros(cfg.hid, np.float32),
                            np.array([cfg.s_pad], np.float32)])[None, :]),
    )
    in_maps = []
    for c in range(cfg.cores):
        m = dict(consts)
        m["x_shard"] = np.ascontiguousarray(x[pp.perms[c]], np.float32)
        m["idx_flat"] = pp.idx_cores[c]
        in_maps.append(m)
    return in_maps


def assemble_output(cfg: GatCfg, pp: Prep, results):
    out = np.zeros((cfg.n, cfg.classes), np.float32)
    for c in range(cfg.cores):
        out[pp.perms[c]] = results[c]["y_out"]
    return out


_cache = {}
last_result = None


def kernel(**inputs) -> np.ndarray:
    global last_result
    cfg = CFG
    trace = bool(int(os.environ.get("GAT_TRACE", "0")))
    if trace:
        _ensure_profile_hook()
    from concourse.bass_utils import run_bass_kernel_spmd

    ei = np.asarray(inputs["edge_index"])
    key = hash(ei.tobytes())
    if key not in _cache:
        pp = preprocess(ei, cfg)
        nc = build_program(cfg, pp)
        _cache[key] = (pp, nc)
    pp, nc = _cache[key]

    in_maps = make_in_maps(
        cfg, pp, inputs["x"], inputs["W0"], inputs["a_s0"], inputs["a_d0"],
        inputs["b0"], inputs["W1"], inputs["a_s1"], inputs["a_d1"],
        inputs["b1"], inputs["Wl"], inputs["bl"])
    res = run_bass_kernel_spmd(nc, in_maps, core_ids=list(range(cfg.cores)),
                               trace=trace)
    last_result = res
    return assemble_output(cfg, pp, res.results)


# revision 16
# speedup vs baseline: 1.0006x; 1.0006x over previous
"""Self-contained 8-core Trainium2 Bass kernel for a 2-layer GAT + linear classifier.

Strategy (dst-sharded 1D graph parallelism):
  - Host: add self-loops, compute in-degrees, sort nodes by degree (desc),
    deal round-robin to 8 cores.  Each core owns 12500 dst nodes; its nodes
    are degree-sorted so 128-node tiles have near-uniform degree.  All
    edge-index-derived tables (gather offset tables, per-tile slot counts)
    are precomputed on the host and baked into the program/inputs.
  - Device, per layer: h' = h @ W and s = h'@a_s are computed per owner shard;
    rows [h' | s] are AllGathered into a full HBM table (100001 x 65, last row
    is a sentinel with h=0, s=-1e4 used for padding slots).  Edges of each dst
    tile are gathered with one indirect DMA (260B per edge slot) into SBUF as
    [128 nodes x D_t slots x 65].  Segment softmax runs along the free axis
    (ACT for exp, DVE for max/recip), the p-weighted aggregation multiplies
    in-place on DVE (broadcast AP) and reduces over slots with PE
    identity-matmul accumulation in PSUM.
  - Output: classifier per tile, unpermuted on the host.
"""

import os
import sys
import types
from dataclasses import dataclass, field

import numpy as np

P = 128


@dataclass
class GatCfg:
    n: int = 100000
    in_dim: int = 128
    hid: int = 64
    classes: int = 40
    cores: int = 8
    slope: float = 0.2
    s_pad: float = -1.0e4
    group_slots: int = 160
    # debug staging: 1=phaseA only, 2=+AG0, 3=+edge L0, 4=full
    stage: int = 4
    use_pe_reduce: bool = False

    @property
    def shard(self):
        assert self.n % self.cores == 0
        return self.n // self.cores

    @property
    def nt(self):
        return (self.shard + P - 1) // P

    @property
    def tab(self):
        return self.n + 1


CFG = GatCfg()


def _ensure_profile_hook():
    """Synthesize antenv.axon_hooks so trace=True can capture NTFF under axon."""
    if "antenv.axon_hooks" in sys.modules:
        return
    try:
        import antenv
        mod = types.ModuleType("antenv.axon_hooks")
        mod._hook = None
        def _set(h):
            mod._hook = h
        def _get():
            return mod._hook
        mod.set_axon_ntff_profile_hook = _set
        mod.get_axon_ntff_profile_hook = _get
        sys.modules["antenv.axon_hooks"] = mod
        antenv.axon_hooks = mod
        from trn_agent_boot.trn_boot import _ntff_profile_via_ctypes
        _set(_ntff_profile_via_ctypes("/opt/axon/libaxon_pjrt.so"))
    except Exception:
        pass


# --------------------------------------------------------------------------
# Host preprocessing: sharding, tiling, gather tables
# --------------------------------------------------------------------------

@dataclass
class Prep:
    perms: list                       # per core: global node ids in local order
    D: np.ndarray                     # [nt] slots per node for each tile
    rows_t: list                      # [nt] rows per tile
    groups: list                      # (t0, n_tiles, S, rows, idx_off)
    idx_cores: list                   # per core: flat int32 gather tables
    tot_idx: int = 0


def preprocess(edge_index, cfg: GatCfg) -> Prep:
    N, NC, GS = cfg.n, cfg.cores, cfg.group_slots
    shard = cfg.shard
    src = np.asarray(edge_index[0]).astype(np.int64)
    dst = np.asarray(edge_index[1]).astype(np.int64)
    loop = np.arange(N, dtype=np.int64)
    src_all = np.concatenate([src, loop])
    dst_all = np.concatenate([dst, loop])
    deg = np.bincount(dst_all, minlength=N).astype(np.int64)
    order = np.argsort(dst_all, kind="stable")
    srcs_by_dst = src_all[order]
    rowptr = np.zeros(N + 1, np.int64)
    np.cumsum(deg, out=rowptr[1:])

    rank_order = np.argsort(-deg, kind="stable")
    table_row = np.empty(N, np.int64)
    rr = np.arange(N)
    table_row[rank_order] = (rr % NC) * shard + rr // NC
    perms = [rank_order[c::NC] for c in range(NC)]

    nt = cfg.nt
    rows_t = [min(P, shard - t * P) for t in range(nt)]
    D = np.zeros(nt, np.int64)
    for c in range(NC):
        dc = deg[perms[c]]
        for t in range(nt):
            D[t] = max(D[t], dc[t * P:t * P + rows_t[t]].max())

    groups = []
    t = 0
    idx_off = 0
    while t < nt:
        if rows_t[t] < P:
            groups.append((t, 1, int(D[t]), rows_t[t], idx_off))
            idx_off += rows_t[t] * int(D[t])
            t += 1
            continue
        S = 0
        t0 = t
        while t < nt and rows_t[t] == P and (S == 0 or S + D[t] <= GS):
            S += int(D[t])
            t += 1
        groups.append((t0, t - t0, S, P, idx_off))
        idx_off += P * S
    tot_idx = idx_off

    SENT = N
    idx_cores = []
    for c in range(NC):
        parts = []
        for (t0, ntl, S, rows, _off) in groups:
            arr = np.full((rows, S), SENT, np.int32)
            off = 0
            for t2 in range(t0, t0 + ntl):
                Dt = int(D[t2])
                nodes = perms[c][t2 * P:t2 * P + rows]
                degs = deg[nodes]
                starts = rowptr[nodes]
                pos = starts[:, None] + np.arange(Dt)[None, :]
                mask = np.arange(Dt)[None, :] < degs[:, None]
                block = np.full((rows, Dt), SENT, np.int64)
                block[mask] = table_row[srcs_by_dst[pos[mask]]]
                arr[:, off:off + Dt] = block.astype(np.int32)
                off += Dt
            parts.append(arr.reshape(-1))
        idx_cores.append(np.ascontiguousarray(np.concatenate(parts), dtype=np.int32))
        assert idx_cores[-1].size == tot_idx
    return Prep(perms=perms, D=D, rows_t=rows_t, groups=groups,
                idx_cores=idx_cores, tot_idx=tot_idx)


# --------------------------------------------------------------------------
# Device program
# --------------------------------------------------------------------------

def build_program(cfg: GatCfg, pp: Prep, enable_asserts=False):
    import concourse.bass as bass
    import concourse.mybir as mybir
    import concourse.tile as tile
    from concourse import bacc

    f32 = mybir.dt.float32
    i32 = mybir.dt.int32
    A = mybir.AluOpType
    AF = mybir.ActivationFunctionType
    IN, H, CLS, NC = cfg.in_dim, cfg.hid, cfg.classes, cfg.cores
    HS = H + 1
    N, NT, TAB, shard = cfg.n, cfg.nt, cfg.tab, cfg.shard
    Dmax = int(pp.D.max())
    GSmax = max(g[2] for g in pp.groups)

    nc = bacc.Bacc("TRN2", target_bir_lowering=False, debug=False,
                   enable_asserts=enable_asserts, num_devices=NC)

    x_t = nc.dram_tensor("x_shard", [shard, IN], f32, kind="ExternalInput")
    idx_t = nc.dram_tensor("idx_flat", [pp.tot_idx], i32, kind="ExternalInput")
    W0_t = nc.dram_tensor("W0", [IN, H], f32, kind="ExternalInput")
    W1_t = nc.dram_tensor("W1", [H, H], f32, kind="ExternalInput")
    Wl_t = nc.dram_tensor("Wl", [H, CLS], f32, kind="ExternalInput")
    asb0_t = nc.dram_tensor("asb0", [P, H], f32, kind="ExternalInput")
    adb0_t = nc.dram_tensor("adb0", [P, H], f32, kind="ExternalInput")
    asb1_t = nc.dram_tensor("asb1", [P, H], f32, kind="ExternalInput")
    adb1_t = nc.dram_tensor("adb1", [P, H], f32, kind="ExternalInput")
    b0b_t = nc.dram_tensor("b0b", [P, H], f32, kind="ExternalInput")
    b1b_t = nc.dram_tensor("b1b", [P, H], f32, kind="ExternalInput")
    blb_t = nc.dram_tensor("blb", [P, CLS], f32, kind="ExternalInput")
    ident_t = nc.dram_tensor("ident", [P, P], f32, kind="ExternalInput")
    sent_t = nc.dram_tensor("sentrow", [1, HS], f32, kind="ExternalInput")
    y_t = nc.dram_tensor("y_out", [shard, CLS], f32, kind="ExternalOutput")

    ha0_loc = nc.dram_tensor("ha0_loc", [shard, HS], f32, kind="Internal")
    ha1_loc = nc.dram_tensor("ha1_loc", [shard, HS], f32, kind="Internal")
    ha0_full = nc.dram_tensor("ha0_full", [TAB, HS], f32, kind="Internal")
    ha1_full = nc.dram_tensor("ha1_full", [TAB, HS], f32, kind="Internal")

    with tile.TileContext(nc) as tc:
        with tc.tile_pool(name="const", bufs=1) as cp, \
             tc.tile_pool(name="gp", bufs=3) as gp, \
             tc.tile_pool(name="wp", bufs=3) as wp, \
             tc.tile_pool(name="ps", bufs=2, space="PSUM") as ps:

            def load_const(t, shape):
                s = cp.tile(shape, f32, tag=f"c_{t.name}")
                nc.sync.dma_start(s[:], t.ap())
                return s

            W0_s = load_const(W0_t, [IN, H])
            W1_s = load_const(W1_t, [H, H])
            Wl_s = load_const(Wl_t, [H, CLS])
            asb0_s = load_const(asb0_t, [P, H])
            adb0_s = load_const(adb0_t, [P, H])
            asb1_s = load_const(asb1_t, [P, H])
            adb1_s = load_const(adb1_t, [P, H])
            b0b_s = load_const(b0b_t, [P, H])
            b1b_s = load_const(b1b_t, [P, H])
            blb_s = load_const(blb_t, [P, CLS])
            ident_s = load_const(ident_t, [P, P])
            sent_s = load_const(sent_t, [1, HS])

            d0_all = cp.tile([P, NT], f32)
            d0s_all = cp.tile([P, NT], f32)
            d1_all = cp.tile([P, NT], f32)
            d1s_all = cp.tile([P, NT], f32)
            junk = cp.tile([P, H], f32)

            # sentinel rows of both tables
            nc.sync.dma_start(ha0_full.ap()[N:N + 1, :], sent_s[:])
            nc.sync.dma_start(ha1_full.ap()[N:N + 1, :], sent_s[:])

            # ---------------- phase A: h0' = x @ W0, s0, d0 ----------------
            for t in range(NT):
                rows = pp.rows_t[t]
                x_s = wp.tile([rows, IN], f32, tag="xs")
                nc.sync.dma_start(x_s[:], x_t.ap()[t * P:t * P + rows, :])
                xT_p = ps.tile([IN, P], f32, space="PSUM", tag="tp")
                nc.tensor.transpose(out=xT_p[:, :rows], in_=x_s[:],
                                    identity=ident_s[:rows, :rows])
                xT_s = wp.tile([IN, P], f32, tag="xts")
                nc.vector.tensor_copy(out=xT_s[:, :rows], in_=xT_p[:, :rows])
                h0_p = ps.tile([P, H], f32, space="PSUM", tag="mm")
                nc.tensor.matmul(out=h0_p[:rows, :], lhsT=xT_s[:, :rows],
                                 rhs=W0_s[:], start=True, stop=True)
                ha_tile = wp.tile([rows, HS], f32, tag="hat")
                nc.scalar.copy(ha_tile[:, :H], h0_p[:rows, :])
                nc.vector.tensor_tensor(out=junk[:rows, :], in0=h0_p[:rows, :],
                                        in1=asb0_s[:rows, :], op=A.mult)
                nc.vector.tensor_reduce(out=ha_tile[:, H:HS], in_=junk[:rows, :],
                                        axis=mybir.AxisListType.X, op=A.add)
                nc.vector.tensor_tensor(out=junk[:rows, :], in0=h0_p[:rows, :],
                                        in1=adb0_s[:rows, :], op=A.mult)
                nc.vector.tensor_reduce(out=d0_all[:rows, t:t + 1],
                                        in_=junk[:rows, :],
                                        axis=mybir.AxisListType.X, op=A.add)
                nc.sync.dma_start(ha0_loc.ap()[t * P:t * P + rows, :], ha_tile[:])
            nc.vector.tensor_scalar(d0s_all[:], d0_all[:], cfg.slope, None,
                                    op0=A.mult)

            if cfg.stage >= 2:
                nc.gpsimd.collective_compute(
                    "AllGather", A.bypass,
                    replica_groups=[list(range(NC))],
                    ins=[ha0_loc.ap()],
                    outs=[ha0_full.ap()[0:N, :]],
                )

            # ---------------- edge phase ----------------
            def edge_phase(table, d_all, ds_all, post):
                for (t0, ntl, S, rows, idx_off) in pp.groups:
                    idx_s = gp.tile([rows, S], i32, tag="idx")
                    nc.sync.dma_start(
                        idx_s[:],
                        idx_t.ap()[idx_off:idx_off + rows * S]
                        .rearrange("(p s) -> p s", s=S))
                    G = gp.tile([rows, S, HS], f32, tag="G")
                    # HW indirect DMA consumes ONE offset per partition per
                    # call (sim's multi-offset form diverges on silicon), so
                    # gather one slot-column (<=128 rows) at a time.
                    for j in range(S):
                        nc.gpsimd.indirect_dma_start(
                            out=G[:, j, :], out_offset=None,
                            in_=table.ap(),
                            in_offset=bass.IndirectOffsetOnAxis(
                                ap=idx_s[:, j:j + 1], axis=0))
                    off = 0
                    for t in range(t0, t0 + ntl):
                        Dt = int(pp.D[t])
                        Gt = G[:, off:off + Dt, :]
                        off += Dt
                        sG = Gt[:, :, H]
                        z = wp.tile([rows, Dt], f32, tag="z")
                        nc.scalar.activation(z[:], sG, AF.Identity,
                                             bias=d_all[:rows, t:t + 1], scale=1.0)
                        t02 = wp.tile([rows, Dt], f32, tag="t02")
                        nc.scalar.activation(t02[:], sG, AF.Identity,
                                             bias=ds_all[:rows, t:t + 1],
                                             scale=cfg.slope)
                        nc.vector.tensor_tensor(out=z[:], in0=z[:], in1=t02[:],
                                                op=A.max)
                        nm = wp.tile([rows, 1], f32, tag="nm")
                        nc.vector.tensor_reduce(out=nm[:], in_=z[:],
                                                axis=mybir.AxisListType.X,
                                                op=A.max, negate=True)
                        p_t = wp.tile([rows, Dt], f32, tag="pt")
                        den = wp.tile([rows, 1], f32, tag="den")
                        nc.scalar.activation(p_t[:], z[:], AF.Exp, bias=nm[:],
                                             scale=1.0, accum_out=den[:])
                        r_t = wp.tile([rows, 1], f32, tag="rt")
                        nc.vector.reciprocal(r_t[:], den[:])
                        nc.vector.tensor_tensor(
                            out=Gt[:, :, 0:H], in0=Gt[:, :, 0:H],
                            in1=p_t[:].to_broadcast([rows, Dt, H]), op=A.mult)
                        hagg = wp.tile([rows, H], f32, tag="hagg")
                        if cfg.use_pe_reduce:
                            agg_p = ps.tile([P, H], f32, space="PSUM", tag="agg")
                            for d in range(Dt):
                                nc.tensor.matmul(out=agg_p[:rows, :],
                                                 lhsT=ident_s[:rows, :rows],
                                                 rhs=Gt[:, d, 0:H],
                                                 start=(d == 0),
                                                 stop=(d == Dt - 1))
                            nc.vector.tensor_scalar(hagg[:], agg_p[:rows, :],
                                                    r_t[:], None, op0=A.mult)
                        else:
                            agg_s = wp.tile([rows, H], f32, tag="aggs")
                            nc.vector.tensor_reduce(
                                out=agg_s[:],
                                in_=Gt[:, :, 0:H].rearrange("p d f -> p f d"),
                                axis=mybir.AxisListType.X, op=A.add)
                            nc.vector.tensor_scalar(hagg[:], agg_s[:],
                                                    r_t[:], None, op0=A.mult)
                        bias_s = b0b_s if table is ha0_full else b1b_s
                        nc.vector.tensor_tensor(out=hagg[:], in0=hagg[:],
                                                in1=bias_s[:rows, :], op=A.add)
                        # ELU = relu(x) + min(exp(x)-1, 0)
                        ex = wp.tile([rows, H], f32, tag="ex")
                        nc.scalar.activation(ex[:], hagg[:], AF.Exp)
                        nc.vector.tensor_scalar(ex[:], ex[:], -1.0, 0.0,
                                                op0=A.add, op1=A.min)
                        rl = wp.tile([rows, H], f32, tag="rl")
                        nc.vector.tensor_scalar(rl[:], hagg[:], 0.0, None,
                                                op0=A.max)
                        h_t = wp.tile([rows, H], f32, tag="ht")
                        nc.vector.tensor_tensor(out=h_t[:], in0=rl[:], in1=ex[:],
                                                op=A.add)
                        post(t, h_t, rows)

            # ---------------- layer-0 post: h1' = elu_out @ W1, s1, d1 -----
            def post_l0(t, h_t, rows):
                hT_p = ps.tile([H, P], f32, space="PSUM", tag="tp")
                nc.tensor.transpose(out=hT_p[:, :rows], in_=h_t[:],
                                    identity=ident_s[:rows, :rows])
                hT_s = wp.tile([H, P], f32, tag="hts")
                nc.vector.tensor_copy(out=hT_s[:, :rows], in_=hT_p[:, :rows])
                h1_p = ps.tile([P, H], f32, space="PSUM", tag="mm")
                nc.tensor.matmul(out=h1_p[:rows, :], lhsT=hT_s[:, :rows],
                                 rhs=W1_s[:], start=True, stop=True)
                ha_tile = wp.tile([rows, HS], f32, tag="hat")
                nc.scalar.copy(ha_tile[:, :H], h1_p[:rows, :])
                nc.vector.tensor_tensor(out=junk[:rows, :], in0=h1_p[:rows, :],
                                        in1=asb1_s[:rows, :], op=A.mult)
                nc.vector.tensor_reduce(out=ha_tile[:, H:HS], in_=junk[:rows, :],
                                        axis=mybir.AxisListType.X, op=A.add)
                nc.vector.tensor_tensor(out=junk[:rows, :], in0=h1_p[:rows, :],
                                        in1=adb1_s[:rows, :], op=A.mult)
                nc.vector.tensor_reduce(out=d1_all[:rows, t:t + 1],
                                        in_=junk[:rows, :],
                                        axis=mybir.AxisListType.X, op=A.add)
                nc.sync.dma_start(ha1_loc.ap()[t * P:t * P + rows, :], ha_tile[:])

            if cfg.stage >= 3:
                edge_phase(ha0_full, d0_all, d0s_all, post_l0)
            if cfg.stage >= 4:
                nc.vector.tensor_scalar(d1s_all[:], d1_all[:], cfg.slope, None,
                                        op0=A.mult)
                nc.gpsimd.collective_compute(
                    "AllGather", A.bypass,
                    replica_groups=[list(range(NC))],
                    ins=[ha1_loc.ap()],
                    outs=[ha1_full.ap()[0:N, :]],
                )

            # ---------------- layer-1 post: classifier ----------------
            def post_l1(t, h_t, rows):
                hT_p = ps.tile([H, P], f32, space="PSUM", tag="tp")
                nc.tensor.transpose(out=hT_p[:, :rows], in_=h_t[:],
                                    identity=ident_s[:rows, :rows])
                hT_s = wp.tile([H, P], f32, tag="hts")
                nc.vector.tensor_copy(out=hT_s[:, :rows], in_=hT_p[:, :rows])
                y_p = ps.tile([P, CLS], f32, space="PSUM", tag="mm")
                nc.tensor.matmul(out=y_p[:rows, :], lhsT=hT_s[:, :rows],
                                 rhs=Wl_s[:], start=True, stop=True)
                y_s = wp.tile([rows, CLS], f32, tag="ys")
                nc.vector.tensor_tensor(out=y_s[:], in0=y_p[:rows, :],
                                        in1=blb_s[:rows, :], op=A.add)
                nc.sync.dma_start(y_t.ap()[t * P:t * P + rows, :], y_s[:])

            if cfg.stage >= 4:
                edge_phase(ha1_full, d1_all, d1s_all, post_l1)
            else:
                for t in range(NT):
                    rows = pp.rows_t[t]
                    nc.sync.dma_start(y_t.ap()[t * P:t * P + rows, :],
                                      blb_s[:rows, :])

    nc.compile()
    return nc


def make_in_maps(cfg: GatCfg, pp: Prep, x, W0, a_s0, a_d0, b0, W1, a_s1, a_d1,
                 b1, Wl, bl):
    x = np.asarray(x, np.float32)
    consts = dict(
        W0=np.ascontiguousarray(W0, np.float32),
        W1=np.ascontiguousarray(W1, np.float32),
        Wl=np.ascontiguousarray(Wl, np.float32),
        asb0=np.ascontiguousarray(np.tile(np.asarray(a_s0, np.float32)[None, :], (P, 1))),
        adb0=np.ascontiguousarray(np.tile(np.asarray(a_d0, np.float32)[None, :], (P, 1))),
        asb1=np.ascontiguousarray(np.tile(np.asarray(a_s1, np.float32)[None, :], (P, 1))),
        adb1=np.ascontiguousarray(np.tile(np.asarray(a_d1, np.float32)[None, :], (P, 1))),
        b0b=np.ascontiguousarray(np.tile(np.asarray(b0, np.float32)[None, :], (P, 1))),
        b1b=np.ascontiguousarray(np.tile(np.asarray(b1, np.float32)[None, :], (P, 1))),
        blb=np.ascontiguousarray(np.tile(np.asarray(bl, np.float32)[None, :], (P, 1))),
        ident=np.eye(P, dtype=np.float32),
        sentrow=np.ascontiguousarray(
            np.concatenate([np.zeros(cfg.hid, np.float32),
                            np.array([cfg.s_pad], np.float32)])[None, :]),
    )
    in_maps = []
    for c in range(cfg.cores):
        m = dict(consts)
        m["x_shard"] = np.ascontiguousarray(x[pp.perms[c]], np.float32)
        m["idx_flat"] = pp.idx_cores[c]
        in_maps.append(m)
    return in_maps


def assemble_output(cfg: GatCfg, pp: Prep, results):
    out = np.zeros((cfg.n, cfg.classes), np.float32)
    for c in range(cfg.cores):
        out[pp.perms[c]] = results[c]["y_out"]
    return out


_cache = {}
last_result = None


def kernel(**inputs) -> np.ndarray:
    global last_result
    cfg = CFG
    trace = bool(int(os.environ.get("GAT_TRACE", "0")))
    if trace:
        _ensure_profile_hook()
    from concourse.bass_utils import run_bass_kernel_spmd

    ei = np.asarray(inputs["edge_index"])
    key = hash(ei.tobytes())
    if key not in _cache:
        pp = preprocess(ei, cfg)
        nc = build_program(cfg, pp)
        _cache[key] = (pp, nc)
    pp, nc = _cache[key]

    in_maps = make_in_maps(
        cfg, pp, inputs["x"], inputs["W0"], inputs["a_s0"], inputs["a_d0"],
        inputs["b0"], inputs["W1"], inputs["a_s1"], inputs["a_d1"],
        inputs["b1"], inputs["Wl"], inputs["bl"])
    res = run_bass_kernel_spmd(nc, in_maps, core_ids=list(range(cfg.cores)),
                               trace=trace)
    last_result = res
    return assemble_output(cfg, pp, res.results)
